# revision 28
# baseline (speedup 1.0000x reference)
"""Trainium2 Bass kernel for nn_ConversationLSTM.

Strategy (data-parallel over batch, per sharding hint):
- 8 cores; core c owns batch rows [32c, 32c+32) of all three text streams,
  fused into S=96 sequences per core.
- All 3 LSTM layers run in ONE software-pipelined scan: at slot u, layer l
  processes step t=u-l. Per layer-step, the gate pre-activations are built
  directly in PSUM: [bias-inject K=1 matmul when b!=0] + x@Wx (x^T stationary)
  + h@Wh (h^T stationary), all bf16 matmuls with fp32 PSUM accumulation.
  Layer 0's x comes from a per-step indirect-DMA embedding gather (cast to
  bf16 in flight); layers 1/2 read the transposed h of the layer below
  straight from SBUF. No DRAM staging of gates or activations at all.
- This keeps TensorE saturated (the three layers' serial chains interleave),
  avoiding the per-step PE idle gaps + HAM cold-clock penalty that dominated
  the phase-split variant.
- The tiny BN/dense head runs as a second, single-core launch in fp32
  (feature-major layout so BN stats are free-dim reductions).
"""

import numpy as np

B, T_FULL, H, V, L = 256, 256, 512, 32000, 3
NCORES = 8
BS = B // NCORES          # 32 batch rows per core
S = 3 * BS                # 96 fused sequences per core
P = 128
HK = H // P               # 4 k-tiles over the hidden dim
G4 = 4 * H                # 2048 gate columns
NB = 4                    # gate banks of 512
D1, D2, D3 = 3 * H, 2 * H, H // 5   # 1536, 1024, 102
SELU_L = 1.0507009873554804934193349852946
SELU_A = 1.6732632423543772848170429916717

_CACHE = {}

# fp8 path: x^T/h^T stationaries and weights in fp8e4 with DoubleRow matmuls
# (2 k-tiles per MM). Scales: activations x16, weights x64 -> PSUM holds
# 1024*gates; undone for free via the activation `scale` argument.
FP8 = False  # fp8e4 DoubleRow measured rel_err ~7e-2 (>2e-2 budget) — keep bf16
XSCALE = 16.0
WSCALE = 64.0
PSCALE = XSCALE * WSCALE


def _build_lstm(T, has_bias, fp8):
    import concourse.bass as bass
    import concourse.tile as tile
    from concourse import bacc, mybir
    from concourse.masks import make_identity
    from contextlib import ExitStack

    fp32, bf16, i32 = mybir.dt.float32, mybir.dt.bfloat16, mybir.dt.int32
    fp8e4 = mybir.dt.float8e4
    sdt = fp8e4 if fp8 else bf16           # stationary / weight dtype
    DR = mybir.MatmulPerfMode.DoubleRow if fp8 else None
    KSTEP = 2 if fp8 else 1
    ascale = (1.0 / PSCALE) if fp8 else 1.0
    AF = mybir.ActivationFunctionType

    nc = bacc.Bacc("TRN2", target_bir_lowering=False, debug=False,
                   num_devices=NCORES)
    idx = nc.dram_tensor("idx", [S, T], i32, kind="ExternalInput").ap()
    emb = nc.dram_tensor("emb", [V, H], fp32, kind="ExternalInput").ap()
    w_d = nc.dram_tensor("w", [P, L, 2, HK, G4], sdt, kind="ExternalInput").ap()
    if has_bias:
        b_d = nc.dram_tensor("b", [L, G4], bf16, kind="ExternalInput").ap()
    hout = nc.dram_tensor("hout", [S, H], fp32, kind="ExternalOutput").ap()

    with tile.TileContext(nc) as tc, ExitStack() as ctx:
        ep = ctx.enter_context
        const_p = ep(tc.tile_pool(name="const", bufs=1))
        xg_p = ep(tc.tile_pool(name="xg", bufs=3))
        xt_p = ep(tc.tile_pool(name="xt", bufs=2))
        st_p = ep(tc.tile_pool(name="st", bufs=3))
        act_p = ep(tc.tile_pool(name="act", bufs=8))
        cell_p = ep(tc.tile_pool(name="cell", bufs=2))
        tmp_p = ep(tc.tile_pool(name="tmp", bufs=4))
        ps_g = ep(tc.tile_pool(name="psg", bufs=6, space="PSUM"))
        ps_t = ep(tc.tile_pool(name="pst", bufs=2, space="PSUM"))

        identb = const_p.tile([S, S], bf16)
        identf = const_p.tile([S, S], fp32)
        make_identity(nc, identf[:])
        nc.vector.tensor_copy(identb[:], identf[:])

        idx_sb = const_p.tile([S, T], i32)
        nc.sync.dma_start(idx_sb[:], idx[:])

        wsb = const_p.tile([P, L, 2, HK, G4], sdt)
        nc.sync.dma_start(wsb[:], w_d[:])

        if has_bias:
            b_sb = const_p.tile([1, L, G4], bf16)
            nc.sync.dma_start(b_sb[:], b_d[:].rearrange("l g -> () l g"))
            ones = const_p.tile([1, S], bf16)
            nc.vector.memset(ones[:], 1.0)

        c_prev = [None] * L
        st_prev = [None] * L
        # f first so fc is off the tail; o last so the post-matmul tail is
        # just ACT(o) -> mul(h), short enough to hide under the other
        # layers' transposes at the slot boundary.
        GATE_ORDER = (1, 2, 0, 3)          # f, g, i, o
        GATE_FN = {0: AF.Sigmoid, 1: AF.Sigmoid, 2: AF.Tanh, 3: AF.Sigmoid}

        transp_q = []  # transposes deferred to slot end: (l, h_bf)

        def emit_transposes(l, h_bf):
            # PE transposes pipeline at ~50ns spacing (~1us/slot total);
            # DMA xbar transposes measured 1.24us EACH on the Sync ring.
            tph = ps_t.tile([P, HK, S], bf16, space="PSUM", tag="tph")
            for k in range(HK):
                nc.tensor.transpose(tph[:, k, :],
                                    h_bf[:, k * P:(k + 1) * P], identb[:])
            st = st_p.tile([P, HK, S], sdt, tag=f"st{l}")
            nc.vector.tensor_copy(st[:], tph[:])
            st_prev[l] = st

        def flush_transposes():
            for l, h_bf in transp_q:
                emit_transposes(l, h_bf)
            del transp_q[:]

        def layer_step(l, t):
            last_l = l == L - 1
            last_t = t == T - 1

            # ---- x^T stationary ----
            if l == 0:
                xg = xg_p.tile([S, H], bf16, tag="xg")
                nc.gpsimd.indirect_dma_start(
                    out=xg[:], out_offset=None, in_=emb[:],
                    in_offset=bass.IndirectOffsetOnAxis(
                        ap=idx_sb[:, t:t + 1], axis=0))
                tpx = ps_t.tile([P, HK, S], bf16, space="PSUM", tag="tph")
                for k in range(HK):
                    nc.tensor.transpose(tpx[:, k, :],
                                        xg[:, k * P:(k + 1) * P], identb[:])
                xT = xt_p.tile([P, HK, S], sdt, tag="xt0")
                nc.vector.tensor_copy(xT[:], tpx[:])
            else:
                xT = st_prev[l - 1]

            # ---- gates: PSUM accumulation [bias] + x@Wx + h@Wh ----
            hT = st_prev[l] if t > 0 else None
            gp = {}
            for n in GATE_ORDER:
                nsl = slice(n * 512, (n + 1) * 512)
                g = ps_g.tile([S, 512], fp32, space="PSUM", tag="gp")
                first = True
                if has_bias:
                    nc.tensor.matmul(g[:], lhsT=ones[:],
                                     rhs=b_sb[:, l, nsl],
                                     start=True, stop=False)
                    first = False
                for k in range(0, HK, KSTEP):
                    ks = slice(k, k + KSTEP) if fp8 else k
                    nc.tensor.matmul(g[:], lhsT=xT[:, ks, :],
                                     rhs=wsb[:, l, 0, ks, nsl],
                                     start=first, perf_mode=DR,
                                     stop=(hT is None and k + KSTEP >= HK))
                    first = False
                if hT is not None:
                    for k in range(0, HK, KSTEP):
                        ks = slice(k, k + KSTEP) if fp8 else k
                        nc.tensor.matmul(g[:], lhsT=hT[:, ks, :],
                                         rhs=wsb[:, l, 1, ks, nsl],
                                         start=False, perf_mode=DR,
                                         stop=(k + KSTEP >= HK))
                gp[n] = g

            # ---- activations (ScalarE, PSUM source) ----
            # f/g/i first; tanh(c) is emitted BEFORE the o-gate ACT so it
            # runs during the o-bank matmuls — the post-matmul tail is then
            # just ACT(o) -> mul(h) (~0.7us/slot shorter).
            a = {}
            for n in GATE_ORDER[:-1]:
                an = act_p.tile([S, 512], fp32, tag="act")
                nc.scalar.activation(an[:], gp[n][:], GATE_FN[n], scale=ascale)
                a[n] = an

            # ---- cell math (DVE fp32) ----
            c_new = cell_p.tile([S, H], fp32, tag=f"c{l}")
            if t == 0:
                nc.vector.tensor_mul(c_new[:], a[0][:], a[2][:])
            else:
                ig = tmp_p.tile([S, H], fp32, tag="ig")
                nc.vector.tensor_mul(ig[:], a[0][:], a[2][:])
                fc = tmp_p.tile([S, H], fp32, tag="fc")
                nc.vector.tensor_mul(fc[:], a[1][:], c_prev[l][:])
                nc.vector.tensor_add(c_new[:], ig[:], fc[:])
            c_prev[l] = c_new
            tc_t = tmp_p.tile([S, H], fp32, tag="tc")
            nc.scalar.activation(tc_t[:], c_new[:], AF.Tanh)

            n = GATE_ORDER[-1]
            an = act_p.tile([S, 512], fp32, tag="act")
            nc.scalar.activation(an[:], gp[n][:], GATE_FN[n], scale=ascale)
            a[n] = an

            if last_l and last_t:
                h_f = tmp_p.tile([S, H], fp32, tag="hf")
                nc.vector.tensor_mul(h_f[:], a[3][:], tc_t[:])
                nc.sync.dma_start(hout[:], h_f[:])
                return

            # h_bf holds XSCALE*h on the fp8 path (the scale is folded into
            # the stationary; weights carry WSCALE; ACT scale undoes both).
            h_bf = tmp_p.tile([S, H], bf16, tag="hbf")
            if fp8:
                nc.vector.scalar_tensor_tensor(
                    h_bf[:], a[3][:], XSCALE, tc_t[:],
                    mybir.AluOpType.mult, mybir.AluOpType.mult)
            else:
                nc.vector.tensor_mul(h_bf[:], a[3][:], tc_t[:])
            transp_q.append((l, h_bf))

        # reverse layer order within a slot (layer l+1 reads the previous
        # slot's st of layer l); all h^T transposes flush at slot end so
        # the PE stream never waits on a cell chain mid-slot.
        for u in range(T + L - 1):
            for l in reversed(range(L)):
                t = u - l
                if 0 <= t < T:
                    layer_step(l, t)
            flush_transposes()

    nc.compile()
    return nc


def _build_head():
    import concourse.bass as bass
    import concourse.tile as tile
    from concourse import bacc, mybir
    from concourse.masks import make_identity
    from contextlib import ExitStack

    fp32 = mybir.dt.float32
    AF = mybir.ActivationFunctionType
    OP = mybir.AluOpType
    EPS = 1e-3
    import math
    LNA = math.log(SELU_A)

    nc = bacc.Bacc("TRN2", target_bir_lowering=False, debug=False,
                   num_devices=1)
    r_in = nc.dram_tensor("r", [B, D1], fp32, kind="ExternalInput").ap()
    W1 = nc.dram_tensor("W1", [D1, D2], fp32, kind="ExternalInput").ap()
    W2 = nc.dram_tensor("W2", [D2, D3], fp32, kind="ExternalInput").ap()
    W3 = nc.dram_tensor("W3", [D3, 4], fp32, kind="ExternalInput").ap()
    vecs = {}
    for nm, dim in (("g1", D1), ("beta1", D1), ("bd1", D2),
                    ("g2", D2), ("beta2", D2), ("bd2", D3),
                    ("g3", D3), ("beta3", D3), ("bd3", 4)):
        vecs[nm] = nc.dram_tensor(nm, [1, dim], fp32, kind="ExternalInput").ap()
    oT = nc.dram_tensor("oT", [4, B], fp32, kind="ExternalOutput").ap()

    FT1, FT2 = D1 // P, D2 // P      # 12, 8
    MB = B // P                      # 2 batch tiles

    with tile.TileContext(nc) as tc, ExitStack() as ctx:
        ep = ctx.enter_context
        const_p = ep(tc.tile_pool(name="const", bufs=1))
        big_p = ep(tc.tile_pool(name="big", bufs=1))
        sm_p = ep(tc.tile_pool(name="sm", bufs=4))
        st_p = ep(tc.tile_pool(name="st", bufs=4))
        ps_p = ep(tc.tile_pool(name="ps", bufs=2, space="PSUM"))

        ident = const_p.tile([P, P], fp32)
        make_identity(nc, ident[:])
        eps_c = const_p.tile([P, 1], fp32)
        nc.vector.memset(eps_c[:], EPS)

        def load_vec(nm, dim):
            """[1, dim] -> [P, dim/P] feature-major, or [dim, 1] if dim < P."""
            if dim >= P:
                v = const_p.tile([P, dim // P], fp32, tag="v_" + nm)
                nc.sync.dma_start(v[:], vecs[nm][0:1, :]
                                  .rearrange("o (f p) -> (o p) f", p=P))
            else:
                v = const_p.tile([dim, 1], fp32, tag="v_" + nm)
                nc.sync.dma_start(v[:], vecs[nm][0:1, :]
                                  .rearrange("o d -> (o d) ()"))
            return v

        g1, b1 = load_vec("g1", D1), load_vec("beta1", D1)
        g2, b2 = load_vec("g2", D2), load_vec("beta2", D2)
        g3, b3 = load_vec("g3", D3), load_vec("beta3", D3)
        bd1 = load_vec("bd1", D2)
        bd2 = load_vec("bd2", D3)
        bd3 = load_vec("bd3", 4)

        def bn_inplace(xT, ftiles, parts, g_sb, be_sb):
            """x feature-major [parts, ftiles, B]; BN over free dim."""
            for f in range(ftiles):
                x = xT[:, f, :] if ftiles > 1 else xT[:, :]
                m = st_p.tile([parts, 1], fp32, tag="m")
                nc.vector.tensor_reduce(m[:], x, mybir.AxisListType.X, OP.add)
                nc.vector.tensor_scalar(m[:], m[:], 1.0 / B, None, OP.mult)
                sq = st_p.tile([parts, B], fp32, tag="sq")
                ssq = st_p.tile([parts, 1], fp32, tag="ssq")
                nc.scalar.activation(sq[:], x, AF.Square, accum_out=ssq[:])
                # v = ssq/B - m^2 ; std = sqrt(v + eps); s = g/std
                msq = st_p.tile([parts, 1], fp32, tag="msq")
                nc.vector.tensor_mul(msq[:], m[:], m[:])
                v = st_p.tile([parts, 1], fp32, tag="v")
                nc.vector.scalar_tensor_tensor(v[:], ssq[:], 1.0 / B, msq[:],
                                               OP.mult, OP.subtract)
                std = st_p.tile([parts, 1], fp32, tag="std")
                nc.scalar.activation(std[:], v[:], AF.Sqrt, bias=eps_c[:parts, :])
                inv = st_p.tile([parts, 1], fp32, tag="inv")
                nc.vector.reciprocal(inv[:], std[:])
                sc = st_p.tile([parts, 1], fp32, tag="sc")
                nc.vector.tensor_mul(sc[:], inv[:],
                                     g_sb[:, f:f + 1] if ftiles > 1 else g_sb[:])
                nc.vector.tensor_scalar(x, x, m[:], sc[:],
                                        OP.subtract, OP.mult)
                nc.vector.tensor_scalar(x, x, be_sb[:, f:f + 1]
                                        if ftiles > 1 else be_sb[:],
                                        None, OP.add)

        def selu_from_psum(dst, ps, bd_col):
            """dst = selu(ps + bd); column-bias AP [parts,1]."""
            parts = ps.shape[0]
            e = st_p.tile([parts, B], fp32, tag="selu_e")
            ba = st_p.tile([parts, 1], fp32, tag="selu_b")
            nc.vector.tensor_scalar(ba[:], bd_col, LNA, None, OP.add)
            nc.scalar.activation(e[:], ps, AF.Exp, bias=ba[:])
            r_ = st_p.tile([parts, B], fp32, tag="selu_r")
            nc.vector.tensor_scalar(r_[:], ps, bd_col, 0.0, OP.add, OP.max)
            t1 = st_p.tile([parts, B], fp32, tag="selu_t")
            nc.vector.scalar_tensor_tensor(t1[:], e[:], SELU_A, r_[:],
                                           OP.min, OP.add)
            nc.vector.tensor_scalar(dst, t1[:], SELU_L, SELU_L * SELU_A,
                                    OP.mult, OP.subtract)

        # ---- load r, transpose to feature-major rT [P, FT1, B] ----
        rT = big_p.tile([P, FT1, B], fp32, tag="rT")
        for mb in range(MB):
            rsb = sm_p.tile([P, D1], fp32, tag="rsb")
            nc.sync.dma_start(rsb[:], r_in[mb * P:(mb + 1) * P, :])
            for f in range(FT1):
                tp = ps_p.tile([P, P], fp32, space="PSUM", tag="tp")
                nc.tensor.transpose(tp[:], rsb[:, f * P:(f + 1) * P], ident[:])
                nc.vector.tensor_copy(rT[:, f, mb * P:(mb + 1) * P], tp[:])

        bn_inplace(rT, FT1, P, g1, b1)

        # ---- dense1 [1536->1024] + selu ----
        w1 = big_p.tile([P, FT1, D2], fp32, tag="w1")
        nc.sync.dma_start(w1[:], W1[:, :].rearrange("(kt p) m -> p kt m", p=P))
        x1 = big_p.tile([P, FT2, B], fp32, tag="x1")
        for mt in range(FT2):
            ps = ps_p.tile([P, B], fp32, space="PSUM", tag="mm1")
            for kt in range(FT1):
                nc.tensor.matmul(ps[:], lhsT=w1[:, kt, mt * P:(mt + 1) * P],
                                 rhs=rT[:, kt, :],
                                 start=(kt == 0), stop=(kt == FT1 - 1))
            selu_from_psum(x1[:, mt, :], ps[:], bd1[:, mt:mt + 1])

        bn_inplace(x1, FT2, P, g2, b2)

        # ---- dense2 [1024->102] + selu ----
        w2 = big_p.tile([P, FT2, D3], fp32, tag="w2")
        nc.sync.dma_start(w2[:], W2[:, :].rearrange("(kt p) m -> p kt m", p=P))
        ps2 = ps_p.tile([D3, B], fp32, space="PSUM", tag="mm2")
        for kt in range(FT2):
            nc.tensor.matmul(ps2[:], lhsT=w2[:, kt, :], rhs=x1[:, kt, :],
                             start=(kt == 0), stop=(kt == FT2 - 1))
        x2 = big_p.tile([D3, B], fp32, tag="x2")
        selu_from_psum(x2[:], ps2[:], bd2[:])

        bn_inplace(x2, 1, D3, g3, b3)

        # ---- dense3 [102->4] ----
        w3 = sm_p.tile([D3, 4], fp32, tag="w3")
        nc.sync.dma_start(w3[:], W3[:, :])
        ps3 = ps_p.tile([4, B], fp32, space="PSUM", tag="mm3")
        nc.tensor.matmul(ps3[:], lhsT=w3[:], rhs=x2[:], start=True, stop=True)
        ob = sm_p.tile([4, B], fp32, tag="ob")
        nc.vector.tensor_scalar(ob[:], ps3[:], bd3[:], None, OP.add)
        nc.sync.dma_start(oT[:], ob[:])

    nc.compile()
    return nc


def _get(key, builder):
    if key not in _CACHE:
        _CACHE[key] = builder()
    return _CACHE[key]


def kernel(text_1, text_2, text_3, emb, Wx, Wh, b,
           g1, beta1, W1, bd1, g2, beta2, W2, bd2, g3, beta3, W3, bd3,
           T_steps=T_FULL, _profile=None):
    import ml_dtypes
    from concourse import bass_utils
    bf16 = ml_dtypes.bfloat16
    _tr = _profile is not None

    T = T_steps
    texts = [np.ascontiguousarray(np.asarray(t)[:, :T], np.int32)
             for t in (text_1, text_2, text_3)]
    emb = np.asarray(emb, np.float32)
    Wx = np.asarray(Wx, np.float32)
    Wh = np.asarray(Wh, np.float32)
    b = np.asarray(b, np.float32)
    has_bias = bool(np.any(b))

    # weights: [L, H, G4] -> [P, L, 2, HK, G4] (x then h on axis 2)
    wdt = ml_dtypes.float8_e4m3 if FP8 else bf16
    wmul = WSCALE if FP8 else 1.0
    wx_t = Wx.reshape(L, HK, P, G4).transpose(2, 0, 1, 3)
    wh_t = Wh.reshape(L, HK, P, G4).transpose(2, 0, 1, 3)
    wxh = np.ascontiguousarray(
        np.stack([wx_t, wh_t], axis=2) * wmul).astype(wdt)  # [P, L, 2, HK, G4]
    emb_dev = np.ascontiguousarray(emb * XSCALE if FP8 else emb, np.float32)

    nc_l = _get(("lstm", T, has_bias, FP8),
                lambda: _build_lstm(T, has_bias, FP8))
    in_maps = []
    for c in range(NCORES):
        tok = np.stack([t[c * BS:(c + 1) * BS, :] for t in texts], 0)  # [3,BS,T]
        idx_c = np.ascontiguousarray(tok.reshape(S, T), np.int32)
        m = {"idx": idx_c, "emb": emb_dev, "w": wxh}
        if has_bias:
            m["b"] = np.ascontiguousarray((b * (PSCALE if FP8 else 1.0))
                                          .astype(bf16))
        in_maps.append(m)
    res = bass_utils.run_bass_kernel_spmd(nc_l, in_maps,
                                          core_ids=list(range(NCORES)),
                                          trace=_tr)
    if _tr:
        _profile["lstm_ns"] = res.exec_time_ns
        _profile["lstm_mean_ns"] = res.mean_exec_time_ns
        _profile["lstm_trace"] = (res.instructions_and_trace or (None, None))[1]
    r = np.empty((B, D1), np.float32)
    for c in range(NCORES):
        h = res.results[c]["hout"]                    # [S, H]
        r[c * BS:(c + 1) * BS, :] = (h.reshape(3, BS, H)
                                     .transpose(1, 0, 2).reshape(BS, D1))

    nc_h = _get(("head",), _build_head)
    hm = {"r": r, "W1": np.ascontiguousarray(W1, np.float32),
          "W2": np.ascontiguousarray(W2, np.float32),
          "W3": np.ascontiguousarray(W3, np.float32)}
    for nm, v in (("g1", g1), ("beta1", beta1), ("bd1", bd1),
                  ("g2", g2), ("beta2", beta2), ("bd2", bd2),
                  ("g3", g3), ("beta3", beta3), ("bd3", bd3)):
        hm[nm] = np.ascontiguousarray(np.asarray(v), np.float32).reshape(1, -1)
    hres = bass_utils.run_bass_kernel_spmd(nc_h, [hm], core_ids=[0], trace=_tr)
    if _tr:
        _profile["head_ns"] = hres.exec_time_ns
        _profile["head_trace"] = (hres.instructions_and_trace or (None, None))[1]
    return np.ascontiguousarray(hres.results[0]["oT"].T)
